# revision 3
# baseline (speedup 1.0000x reference)
"""Causal multi-head self-attention (RoPE) for Trainium2, distributed over 8 NeuronCores.

Sharding strategy (tensor-parallel over heads x data-parallel over batch):
  core c handles batch b = c // 2 and head-group g = c % 2 (8 of 16 heads).
  Each core computes q/k/v projections for its 8 heads on its batch, RoPE,
  block-causal flash-style attention, and the output projection against its
  512 rows of wo -- producing a partial [S, D] output (bf16).  The host-side
  gather sums the two partials per batch (the tensor-parallel reduce) and
  stacks batches to the full [B, S, D] fp32 output.

Device design notes (v2):
  - All matmuls contract on partitions; host feeds x and weights
    pre-transposed (layout only).
  - Startup DMAs are issued from four different engines (sync/scalar/vector/
    gpsimd queues) so descriptor issue (~0.6us each) does not serialize, and
    x-tile/weight loads are interleaved so the first v-projection matmul can
    start as soon as its own operands land.
  - q and k projections for one head-pair share one [128,1024] PSUM tile;
    RoPE is applied to both in one pass of 5 DVE ops in bf16 (evict, pair-
    swap shuffle, 2 muls via a duplicated cos/sin table, add).
  - Scores are computed transposed [keys, queries] with the valid region
    left-aligned per head (head A at cols [0:512-lo], head B at
    [512:1024-lo]), so every diagonal tile's triangular mask lands at fixed
    columns [0:128] and [512:640]: one small tensor_tensor with a broadcast
    triangle applies the mask for both heads.
  - The two heads' PV accumulators share one [128,1024] PSUM tile; the
    softmax denominator comes from a ones-column appended to v (row 64).
    One eviction + one spread-DMA/reciprocal/broadcast chain normalizes
    both heads.  No row-max subtraction is needed because scores are
    ~N(0,1) bounded.
  - The kernel streams 4 sequence chunks of 512; chunk c's q/k projections
    and the previous chunk's attention interleave in the PE stream.
"""

import math
import sys

import numpy as np

if "/opt/trn_rl_repo" not in sys.path:
    sys.path.insert(0, "/opt/trn_rl_repo")

import contextlib

import concourse.bacc as bacc
import concourse.tile as tile
from concourse import mybir
from concourse.bass_interp import get_hw_module
from concourse.bass_utils import run_bass_kernel_spmd


def _ensure_profile_hook():
    """This image's antenv package lacks axon_hooks, which
    run_bass_kernel_spmd imports under BASS_TRACE=1.  Provide the module and,
    when possible, register the real NTFF profiling hook so tracing works."""
    import types
    try:
        import antenv.axon_hooks  # noqa: F401
        return
    except ImportError:
        pass
    import antenv
    mod = types.ModuleType("antenv.axon_hooks")
    _HOOK = [None]
    mod.set_axon_ntff_profile_hook = lambda h: _HOOK.__setitem__(0, h)
    mod.get_axon_ntff_profile_hook = lambda: _HOOK[0]
    sys.modules["antenv.axon_hooks"] = mod
    antenv.axon_hooks = mod
    try:
        from trn_agent_boot.trn_boot import _ntff_profile_via_ctypes
        import os
        so = "/opt/axon/libaxon_pjrt.so"
        if os.path.exists(so):
            mod.set_axon_ntff_profile_hook(_ntff_profile_via_ctypes(so))
        import concourse.bass_utils as _bu
        _orig_upload = _bu.upload_artifacts

        def _safe_upload(tmpdir):
            try:
                return _orig_upload(tmpdir)
            except Exception:
                return f"local:{tmpdir}"

        _bu.upload_artifacts = _safe_upload
    except Exception:
        pass


_ensure_profile_hook()

F32 = mybir.dt.float32
BF16 = mybir.dt.bfloat16
I32 = mybir.dt.int32

B, S, D = 4, 2048, 1024
H, DH = 16, 64
GD = 512           # head dims per core (8 heads)
THETA = 10000.0
SWAP_MASK = [i ^ 1 for i in range(32)]
MUL = mybir.AluOpType.mult
ADD = mybir.AluOpType.add


def _build_program():
    nc = bacc.Bacc("TRN2", target_bir_lowering=False, debug=False,
                   enable_asserts=False, num_devices=8)

    xT = nc.dram_tensor("xT", [D, S], BF16, kind="ExternalInput").ap()
    wqT = nc.dram_tensor("wqT", [D, GD], BF16, kind="ExternalInput").ap()
    wkT = nc.dram_tensor("wkT", [D, GD], BF16, kind="ExternalInput").ap()
    wvT = nc.dram_tensor("wvT", [D, GD], BF16, kind="ExternalInput").ap()
    woT = nc.dram_tensor("woT", [GD, D], BF16, kind="ExternalInput").ap()
    cosd = nc.dram_tensor("cosd", [128, 2 * S], BF16, kind="ExternalInput").ap()
    sind = nc.dram_tensor("sind", [128, 2 * S], BF16, kind="ExternalInput").ap()
    trid = nc.dram_tensor("trid", [128, 256], BF16, kind="ExternalInput").ap()
    outp = nc.dram_tensor("outp", [S, D], BF16, kind="ExternalOutput").ap()

    with tile.TileContext(nc) as tc:
        _body(tc, nc, xT, wqT, wkT, wvT, woT, cosd, sind, trid, outp)
    nc.compile()
    return nc


def _body(tc, nc, xT, wqT, wkT, wvT, woT, cosd, sind, trid, outp):
    ctx = contextlib.ExitStack()

    singles = ctx.enter_context(tc.tile_pool(name="singles", bufs=1))

    # ---- persistent tiles ---------------------------------------------------
    wv_sb = [singles.tile([128, GD], BF16, tag=f"wv{i}", name=f"wv{i}") for i in range(8)]
    wq_sb = [singles.tile([128, GD], BF16, tag=f"wq{i}", name=f"wq{i}") for i in range(8)]
    wk_sb = [singles.tile([128, GD], BF16, tag=f"wk{i}", name=f"wk{i}") for i in range(8)]
    wo_sb = [singles.tile([128, D], BF16, tag=f"wo{i}", name=f"wo{i}") for i in range(4)]
    cosb = singles.tile([128, 2 * S], BF16, tag="cosb", name="cosb")
    sinb = singles.tile([128, 2 * S], BF16, tag="sinb", name="sinb")
    tri = singles.tile([128, 256], BF16, tag="tri", name="tri")
    # qkT[ot]: cols [0:S] = qT of head-pair ot, cols [S:2S] = kT
    qkT = [singles.tile([128, 2 * S], BF16, tag=f"qkT{i}", name=f"qkT{i}") for i in range(4)]
    oT = [singles.tile([128, S], BF16, tag=f"oT{i}", name=f"oT{i}") for i in range(4)]
    vt = [singles.tile([128, 8 * 65], BF16, tag=f"v{i}", name=f"v{i}") for i in range(16)]

    xt_pool = ctx.enter_context(tc.tile_pool(name="xt", bufs=2))

    # ---- startup DMAs: 4 engines issue in parallel --------------------------
    # sync queue: chunk-0 x tiles interleaved with wv (v-projection critical)
    xt0 = []
    for ic in range(8):
        t = xt_pool.tile([128, 512], BF16, tag=f"xt{ic}", name=f"xt0_{ic}")
        nc.sync.dma_start(out=t, in_=xT[ic * 128:(ic + 1) * 128, 0:512])
        nc.sync.dma_start(out=wv_sb[ic], in_=wvT[ic * 128:(ic + 1) * 128, :])
        xt0.append(t)
    # scalar queue: wq/wk interleaved
    for ic in range(8):
        nc.scalar.dma_start(out=wq_sb[ic], in_=wqT[ic * 128:(ic + 1) * 128, :])
        nc.scalar.dma_start(out=wk_sb[ic], in_=wkT[ic * 128:(ic + 1) * 128, :])
    # gpsimd queue: rope tables (split for queue parallelism), wo, triangle
    for i in range(4):
        csl = slice(i * 1024, (i + 1) * 1024)
        nc.gpsimd.dma_start(out=cosb[:, csl], in_=cosd[:, csl])
        nc.gpsimd.dma_start(out=sinb[:, csl], in_=sind[:, csl])
    # sync queue (after xt0/wv): chunk-1 x prefetch
    xt1 = []
    for ic in range(8):
        t = xt_pool.tile([128, 512], BF16, tag=f"xt{ic}", name=f"xt1_{ic}")
        nc.sync.dma_start(out=t, in_=xT[ic * 128:(ic + 1) * 128, 512:1024])
        xt1.append(t)
    for i in range(4):
        nc.gpsimd.dma_start(out=wo_sb[i], in_=woT[i * 128:(i + 1) * 128, :])
    nc.gpsimd.dma_start(out=tri, in_=trid)
    for st in range(16):
        v3 = vt[st].rearrange("p (h c) -> p h c", h=8)
        nc.gpsimd.memset(v3[:, :, 64:65], 1.0)

    # ---- pools --------------------------------------------------------------
    tmp_pool = ctx.enter_context(tc.tile_pool(name="tmp", bufs=2))
    pt_pool = ctx.enter_context(tc.tile_pool(name="pt", bufs=8))
    norm_pool = ctx.enter_context(tc.tile_pool(name="norm", bufs=2))
    ost_pool = ctx.enter_context(tc.tile_pool(name="ost", bufs=2))
    big_ps = ctx.enter_context(tc.tile_pool(name="big_ps", bufs=2, space="PSUM"))
    po_ps = ctx.enter_context(tc.tile_pool(name="po_ps", bufs=2, space="PSUM"))

    cos3 = cosb.rearrange("p (j c) -> p j c", j=2)
    sin3 = sinb.rearrange("p (j c) -> p j c", j=2)
    tri3 = tri.rearrange("p (j c) -> p j c", j=2)

    def r3(ap):
        return ap.rearrange("p (j c) -> p j c", j=2)

    def qk_proj_rope(ot, sc, xt):
        """Project q and k for head-pair ot of chunk sc, apply RoPE (bf16)."""
        ssl = slice(sc * 512, (sc + 1) * 512)
        ps = big_ps.tile([128, 1024], F32, tag="big", name=f"psqk{sc}_{ot}")
        osl = slice(ot * 128, (ot + 1) * 128)
        for ic in range(8):
            nc.tensor.matmul(ps[:, 0:512], wq_sb[ic][:, osl], xt[ic][:],
                             start=(ic == 0), stop=(ic == 7))
            nc.tensor.matmul(ps[:, 512:1024], wk_sb[ic][:, osl], xt[ic][:],
                             start=(ic == 0), stop=(ic == 7))
        xb = tmp_pool.tile([128, 1024], BF16, tag="xb", name="xb")
        nc.vector.tensor_copy(out=xb, in_=ps)
        xs = tmp_pool.tile([128, 1024], BF16, tag="xs", name="xs")
        nc.vector.stream_shuffle(xs[:], xb[:], SWAP_MASK)
        t1 = tmp_pool.tile([128, 1024], BF16, tag="t1", name="t1")
        nc.vector.tensor_tensor(r3(t1), r3(xb), cos3[:, :, ssl], MUL)
        t2 = tmp_pool.tile([128, 1024], BF16, tag="t2", name="t2")
        nc.vector.tensor_tensor(r3(t2), r3(xs), sin3[:, :, ssl], MUL)
        qk_dst = r3(qkT[ot])[:, :, ssl]
        nc.vector.tensor_tensor(qk_dst, r3(t1), r3(t2), ADD)

    def v_proj(xt, sc, on_scalar):
        """v-projection for chunk sc, ic-major so it streams as DMAs land."""
        for half in range(2):
            psv = big_ps.tile([128, 1024], F32, tag="big", name=f"psv{sc}_{half}")
            for ic in range(8):
                for sub in range(2):
                    stl = 2 * half + sub
                    nc.tensor.matmul(psv[:, sub * 512:(sub + 1) * 512],
                                     xt[ic][:, stl * 128:(stl + 1) * 128],
                                     wv_sb[ic][:],
                                     start=(ic == 0), stop=(ic == 7))
            for sub in range(2):
                st = 4 * sc + 2 * half + sub
                v3 = vt[st].rearrange("p (h c) -> p h c", h=8)
                p3 = psv[:, sub * 512:(sub + 1) * 512].rearrange(
                    "p (h c) -> p h c", h=8)
                if on_scalar:
                    nc.scalar.copy(out=v3[:, :, 0:64], in_=p3[:, :, :])
                else:
                    nc.vector.tensor_copy(out=v3[:, :, 0:64], in_=p3[:, :, :])

    def attn(hp, qc, po, kts, first_kt):
        """Score+softmax+PV for head-pair hp, query chunk qc, key tiles kts.

        Valid region is left-aligned: head A scores at cols [0:512-lo],
        head B at [512:1024-lo]; PV writes po cols [lo:512] / [512+lo:1024].
        """
        for kt in kts:
            d = kt - 4 * qc
            lo = 128 * d if d >= 1 else 0
            ksl = slice(S + kt * 128, S + (kt + 1) * 128)
            qsl = slice(qc * 512 + lo, (qc + 1) * 512)
            ps2 = big_ps.tile([128, 1024], F32, tag="big", name="ps2")
            with tc.high_priority(offset=500):
                nc.tensor.matmul(ps2[:, 0:512 - lo], qkT[hp][0:64, ksl],
                                 qkT[hp][0:64, qsl], start=True, stop=True)
                nc.tensor.matmul(ps2[:, 512:1024 - lo], qkT[hp][64:128, ksl],
                                 qkT[hp][64:128, qsl], start=True, stop=True)
                pt = pt_pool.tile([128, 1024], BF16, tag="pt", name="pt")
                nc.scalar.activation(pt[:, 0:1024 - lo], ps2[:, 0:1024 - lo],
                                     mybir.ActivationFunctionType.Exp, scale=0.125)
                if d >= 0:
                    ptd = r3(pt)[:, :, 0:128]
                    nc.vector.tensor_tensor(ptd, ptd, tri3[:, :, :], MUL)
            c0 = (2 * hp) * 65
            c1 = (2 * hp + 1) * 65
            st = (kt == first_kt)
            sp = (kt == kts[-1] and kt == 4 * qc + 3)
            nc.tensor.matmul(po[0:65, lo:512], vt[kt][:, c0:c0 + 65],
                             pt[:, 0:512 - lo], start=st, stop=sp)
            nc.tensor.matmul(po[0:65, 512 + lo:1024], vt[kt][:, c1:c1 + 65],
                             pt[:, 512:1024 - lo], start=st, stop=sp)

    def finish(hp, qc, po):
        """Evict both heads' PV accumulators, normalize by the ones-row."""
        qsl = slice(qc * 512, (qc + 1) * 512)
        otAB = norm_pool.tile([128, 1024], BF16, tag="otAB", name="otAB")
        nc.vector.tensor_copy(out=otAB[0:65, :], in_=po[0:65, :])
        # spread the 1024 l values over 128 partitions for the reciprocal
        lsp = norm_pool.tile([128, 8], BF16, tag="lsp", name="lsp")
        nc.gpsimd.dma_start(out=lsp[:, :], in_=otAB[64:65, :])
        lspr = norm_pool.tile([128, 8], F32, tag="lspr", name="lspr")
        nc.vector.reciprocal(lspr[:, :], lsp[:, :])
        lb = norm_pool.tile([128, 1024], F32, tag="lb", name="lb")
        nc.gpsimd.dma_start(out=lb[0:1, :], in_=lspr[:, :])
        nc.gpsimd.partition_broadcast(lb[0:64, :], lb[0:1, :], 64)
        nc.vector.tensor_tensor(oT[hp][0:64, qsl], otAB[0:64, 0:512],
                                lb[0:64, 0:512], MUL)
        o1 = norm_pool.tile([128, 512], BF16, tag="o1", name="o1")
        nc.vector.tensor_tensor(o1[0:64, :], otAB[0:64, 512:1024],
                                lb[0:64, 512:1024], MUL)
        nc.sync.dma_start(out=oT[hp][64:128, qsl], in_=o1[0:64, :])

    def outproj(qc):
        if qc < 3:
            for stl in range(4):
                st = 4 * qc + stl
                stsl = slice(st * 128, (st + 1) * 128)
                pso = po_ps.tile([128, 1024], F32, tag="po", name="pso")
                for oc in range(2):
                    osl = slice(oc * 512, (oc + 1) * 512)
                    for hp in range(4):
                        nc.tensor.matmul(pso[:, osl], oT[hp][:, stsl],
                                         wo_sb[hp][:, osl],
                                         start=(hp == 0), stop=(hp == 3))
                ost = ost_pool.tile([128, 1024], BF16, tag="ost", name="ost")
                nc.vector.tensor_copy(out=ost, in_=pso)
                nc.sync.dma_start(out=outp[stsl, :], in_=ost[:])
        else:
            # tail: accumulate hp 0..2 first, add hp=3 (late) separately so
            # the in-order PE stream doesn't block on its normalization
            osts = []
            for stl in range(4):
                st = 4 * qc + stl
                stsl = slice(st * 128, (st + 1) * 128)
                pso = po_ps.tile([128, 1024], F32, tag="po", name="pso")
                for oc in range(2):
                    osl = slice(oc * 512, (oc + 1) * 512)
                    for hp in range(3):
                        nc.tensor.matmul(pso[:, osl], oT[hp][:, stsl],
                                         wo_sb[hp][:, osl],
                                         start=(hp == 0), stop=(hp == 2))
                ost = ost_pool.tile([128, 1024], BF16, tag="ost3",
                                    name=f"ost3_{stl}", bufs=4)
                nc.vector.tensor_copy(out=ost, in_=pso)
                osts.append(ost)
            for stl in range(4):
                st = 4 * qc + stl
                stsl = slice(st * 128, (st + 1) * 128)
                ost = osts[stl]
                pso = po_ps.tile([128, 1024], F32, tag="po", name="pso")
                for oc in range(2):
                    osl = slice(oc * 512, (oc + 1) * 512)
                    nc.tensor.matmul(pso[:, osl], oT[3][:, stsl],
                                     wo_sb[3][:, osl], start=True, stop=True)
                nc.vector.tensor_tensor(ost[:], ost[:], pso[:], ADD)
                # split the tail stores across queues to shorten the drain
                for qtr in range(4):
                    rsl = slice(st * 128 + qtr * 32, st * 128 + (qtr + 1) * 32)
                    nc.sync.dma_start(out=outp[rsl, :],
                                      in_=ost[qtr * 32:(qtr + 1) * 32, :])

    # ---- chunk loop ---------------------------------------------------------
    prefetched = [None]
    for sc in range(4):
        qc = sc
        nkt = 4 * qc + 4
        if sc == 0:
            xt = xt0
        elif sc == 1:
            xt = xt1
        else:
            xt = prefetched[0]

        if sc == 0:
            v_proj(xt0, 0, on_scalar=True)
            qk_proj_rope(0, 0, xt0)
            v_proj(xt1, 1, on_scalar=True)
            for ot in range(1, 4):
                qk_proj_rope(ot, 0, xt0)
            for hp in range(4):
                po = po_ps.tile([128, 1024], F32, tag="po", name="po")
                attn(hp, 0, po, list(range(4)), 0)
                finish(hp, 0, po)
        else:
            qk_proj_rope(0, sc, xt)
            po = po_ps.tile([128, 1024], F32, tag="po", name="po")
            attn(0, qc, po, list(range(4 * qc)), 0)
            if sc >= 2:
                v_proj(xt, sc, on_scalar=False)
            qk_proj_rope(1, sc, xt)
            if sc < 3:
                nxt = []
                for ic in range(8):
                    t = xt_pool.tile([128, 512], BF16, tag=f"xt{ic}",
                                     name=f"xtp{sc + 1}_{ic}")
                    nc.sync.dma_start(
                        out=t, in_=xT[ic * 128:(ic + 1) * 128,
                                      (sc + 1) * 512:(sc + 2) * 512])
                    nxt.append(t)
                prefetched[0] = nxt
            attn(0, qc, po, list(range(4 * qc, nkt)), 0)
            finish(0, qc, po)
            qk_proj_rope(2, sc, xt)
            po = po_ps.tile([128, 1024], F32, tag="po", name="po")
            attn(1, qc, po, list(range(nkt)), 0)
            finish(1, qc, po)
            qk_proj_rope(3, sc, xt)
            for hp in range(2, 4):
                po = po_ps.tile([128, 1024], F32, tag="po", name="po")
                attn(hp, qc, po, list(range(nkt)), 0)
                finish(hp, qc, po)
        outproj(qc)

    ctx.close()


_NC_CACHE = []
LAST_RESULT = None


def _get_program():
    if not _NC_CACHE:
        _NC_CACHE.append(_build_program())
    return _NC_CACHE[0]


def _host_tables(pos):
    p = np.arange(128)
    inv = (THETA ** (-2.0 * ((p % 64) // 2) / DH)).astype(np.float64)
    ang = pos.astype(np.float64)[None, :] * inv[:, None]          # [128, S]
    altsign = np.where(p % 2 == 0, -1.0, 1.0)[:, None]
    cosT = np.cos(ang)
    sinT = np.sin(ang) * altsign
    cos2 = np.concatenate([cosT, cosT], axis=1)                   # [128, 2S]
    sin2 = np.concatenate([sinT, sinT], axis=1)
    tri = np.zeros((128, 256), np.float32)
    c = np.arange(128)
    tri[:, 0:128] = (p[:, None] <= c[None, :])
    tri[:, 128:256] = tri[:, 0:128]
    return _bf16(cos2), _bf16(sin2), _bf16(tri)


def _bf16(a):
    import ml_dtypes
    return np.ascontiguousarray(np.asarray(a, dtype=np.float32)).astype(ml_dtypes.bfloat16)


def kernel(x, token_positions, wq, wk, wv, wo):
    x = np.asarray(x, dtype=np.float32)
    pos = np.asarray(token_positions, dtype=np.int32)
    wq = np.asarray(wq, dtype=np.float32)
    wk = np.asarray(wk, dtype=np.float32)
    wv = np.asarray(wv, dtype=np.float32)
    wo = np.asarray(wo, dtype=np.float32)

    nc = _get_program()
    cos2, sin2, tri = _host_tables(pos)

    in_maps = []
    for c in range(8):
        b, g = c // 2, c % 2
        gsl = slice(g * GD, (g + 1) * GD)
        in_maps.append({
            "xT": _bf16(x[b].T),
            "wqT": _bf16(wq.T[:, gsl]),
            "wkT": _bf16(wk.T[:, gsl]),
            "wvT": _bf16(wv.T[:, gsl]),
            "woT": _bf16(wo.T[gsl, :]),
            "cosd": cos2,
            "sind": sin2,
            "trid": tri,
        })

    old_m = nc.m
    nc.m = get_hw_module(nc.m)
    try:
        res = run_bass_kernel_spmd(nc, in_maps, core_ids=list(range(8)))
    finally:
        nc.m = old_m
    global LAST_RESULT
    LAST_RESULT = res

    out = np.empty((B, S, D), dtype=np.float32)
    for b in range(B):
        # tensor-parallel gather: sum the two head-group partials per batch
        out[b] = (res.results[2 * b]["outp"].astype(np.float32)
                  + res.results[2 * b + 1]["outp"].astype(np.float32))
    return out


# revision 8
# speedup vs baseline: 1.1389x; 1.1389x over previous
"""Causal multi-head self-attention (RoPE) for Trainium2, distributed over 8 NeuronCores.

Sharding strategy (tensor-parallel over heads x data-parallel over batch):
  core c handles batch b = c // 2 and head-group g = c % 2 (8 of 16 heads).
  Each core computes q/k/v projections for its 8 heads on its batch, RoPE,
  block-causal flash-style attention, and the output projection against its
  512 rows of wo -- producing a partial [S, D] output (bf16).  The host-side
  gather sums the two partials per batch (the tensor-parallel reduce) and
  stacks batches to the full [B, S, D] fp32 output.

Device design notes (v2):
  - All matmuls contract on partitions; host feeds x and weights
    pre-transposed (layout only).
  - Startup DMAs are issued from four different engines (sync/scalar/vector/
    gpsimd queues) so descriptor issue (~0.6us each) does not serialize, and
    x-tile/weight loads are interleaved so the first v-projection matmul can
    start as soon as its own operands land.
  - q and k projections for one head-pair share one [128,1024] PSUM tile;
    RoPE is applied to both in one pass of 5 DVE ops in bf16 (evict, pair-
    swap shuffle, 2 muls via a duplicated cos/sin table, add).
  - Scores are computed transposed [keys, queries] with the valid region
    left-aligned per head (head A at cols [0:512-lo], head B at
    [512:1024-lo]), so every diagonal tile's triangular mask lands at fixed
    columns [0:128] and [512:640]: one small tensor_tensor with a broadcast
    triangle applies the mask for both heads.
  - The two heads' PV accumulators share one [128,1024] PSUM tile; the
    softmax denominator comes from a ones-column appended to v (row 64).
    One eviction + one spread-DMA/reciprocal/broadcast chain normalizes
    both heads.  No row-max subtraction is needed because scores are
    ~N(0,1) bounded.
  - The kernel streams 4 sequence chunks of 512; chunk c's q/k projections
    and the previous chunk's attention interleave in the PE stream.
"""

import math
import sys

import numpy as np

if "/opt/trn_rl_repo" not in sys.path:
    sys.path.insert(0, "/opt/trn_rl_repo")

import contextlib

import concourse.bacc as bacc
import concourse.tile as tile
from concourse import mybir
from concourse.bass_interp import get_hw_module
from concourse.bass_utils import run_bass_kernel_spmd


def _ensure_profile_hook():
    """This image's antenv package lacks axon_hooks, which
    run_bass_kernel_spmd imports under BASS_TRACE=1.  Provide the module and,
    when possible, register the real NTFF profiling hook so tracing works."""
    import types
    try:
        import antenv.axon_hooks  # noqa: F401
        return
    except ImportError:
        pass
    import antenv
    mod = types.ModuleType("antenv.axon_hooks")
    _HOOK = [None]
    mod.set_axon_ntff_profile_hook = lambda h: _HOOK.__setitem__(0, h)
    mod.get_axon_ntff_profile_hook = lambda: _HOOK[0]
    sys.modules["antenv.axon_hooks"] = mod
    antenv.axon_hooks = mod
    try:
        from trn_agent_boot.trn_boot import _ntff_profile_via_ctypes
        import os
        so = "/opt/axon/libaxon_pjrt.so"
        if os.path.exists(so):
            mod.set_axon_ntff_profile_hook(_ntff_profile_via_ctypes(so))
        import concourse.bass_utils as _bu
        _orig_upload = _bu.upload_artifacts

        def _safe_upload(tmpdir):
            try:
                return _orig_upload(tmpdir)
            except Exception:
                return f"local:{tmpdir}"

        _bu.upload_artifacts = _safe_upload
    except Exception:
        pass


_ensure_profile_hook()

F32 = mybir.dt.float32
BF16 = mybir.dt.bfloat16
I32 = mybir.dt.int32

B, S, D = 4, 2048, 1024
H, DH = 16, 64
GD = 512           # head dims per core (8 heads)
THETA = 10000.0
SWAP_MASK = [i ^ 1 for i in range(32)]
MUL = mybir.AluOpType.mult
ADD = mybir.AluOpType.add


def _build_program():
    nc = bacc.Bacc("TRN2", target_bir_lowering=False, debug=False,
                   enable_asserts=False, num_devices=8)

    xT = nc.dram_tensor("xT", [D, S], BF16, kind="ExternalInput").ap()
    wqT = nc.dram_tensor("wqT", [D, GD], BF16, kind="ExternalInput").ap()
    wkT = nc.dram_tensor("wkT", [D, GD], BF16, kind="ExternalInput").ap()
    wvT = nc.dram_tensor("wvT", [D, GD], BF16, kind="ExternalInput").ap()
    woT = nc.dram_tensor("woT", [GD, D], BF16, kind="ExternalInput").ap()
    cosd = nc.dram_tensor("cosd", [128, 2 * S], BF16, kind="ExternalInput").ap()
    sind = nc.dram_tensor("sind", [128, 2 * S], BF16, kind="ExternalInput").ap()
    trid = nc.dram_tensor("trid", [128, 256], BF16, kind="ExternalInput").ap()
    outp = nc.dram_tensor("outp", [S, D], BF16, kind="ExternalOutput").ap()

    with tile.TileContext(nc) as tc:
        _body(tc, nc, xT, wqT, wkT, wvT, woT, cosd, sind, trid, outp)
    nc.compile()
    return nc


def _body(tc, nc, xT, wqT, wkT, wvT, woT, cosd, sind, trid, outp):
    ctx = contextlib.ExitStack()

    singles = ctx.enter_context(tc.tile_pool(name="singles", bufs=1))

    # ---- persistent tiles ---------------------------------------------------
    wv_sb = [singles.tile([128, GD], BF16, tag=f"wv{i}", name=f"wv{i}") for i in range(8)]
    wq_sb = [singles.tile([128, GD], BF16, tag=f"wq{i}", name=f"wq{i}") for i in range(8)]
    wk_sb = [singles.tile([128, GD], BF16, tag=f"wk{i}", name=f"wk{i}") for i in range(8)]
    wo_sb = [singles.tile([128, D], BF16, tag=f"wo{i}", name=f"wo{i}") for i in range(4)]
    cosb = singles.tile([128, 2 * S], BF16, tag="cosb", name="cosb")
    sinb = singles.tile([128, 2 * S], BF16, tag="sinb", name="sinb")
    tri = singles.tile([128, 256], BF16, tag="tri", name="tri")
    # qkT[ot]: cols [0:S] = qT of head-pair ot, cols [S:2S] = kT
    qkT = [singles.tile([128, 2 * S], BF16, tag=f"qkT{i}", name=f"qkT{i}") for i in range(4)]
    oT = [singles.tile([128, S], BF16, tag=f"oT{i}", name=f"oT{i}") for i in range(4)]
    vt = [singles.tile([128, 8 * 65], BF16, tag=f"v{i}", name=f"v{i}") for i in range(16)]

    xt_pool = ctx.enter_context(tc.tile_pool(name="xt", bufs=2))

    # ---- startup DMAs: 4 engines issue in parallel --------------------------
    # sync queue: chunk-0 x tiles (v-projection critical), then chunk-1 x
    xt0 = []
    for ic in range(8):
        t = xt_pool.tile([128, 512], BF16, tag=f"xt{ic}", name=f"xt0_{ic}")
        nc.sync.dma_start(out=t, in_=xT[ic * 128:(ic + 1) * 128, 0:512])
        xt0.append(t)
    xt1 = []
    for ic in range(8):
        t = xt_pool.tile([128, 512], BF16, tag=f"xt{ic}", name=f"xt1_{ic}")
        nc.sync.dma_start(out=t, in_=xT[ic * 128:(ic + 1) * 128, 512:1024])
        xt1.append(t)
    # scalar queue: wq/wk interleaved
    for ic in range(8):
        nc.scalar.dma_start(out=wq_sb[ic], in_=wqT[ic * 128:(ic + 1) * 128, :])
        nc.scalar.dma_start(out=wk_sb[ic], in_=wkT[ic * 128:(ic + 1) * 128, :])
    # gpsimd queue: wv (pairs with sync's xt0), rope tables, wo, triangle
    for ic in range(8):
        nc.gpsimd.dma_start(out=wv_sb[ic], in_=wvT[ic * 128:(ic + 1) * 128, :])
    for i in range(4):
        csl = slice(i * 1024, (i + 1) * 1024)
        nc.gpsimd.dma_start(out=cosb[:, csl], in_=cosd[:, csl])
        nc.gpsimd.dma_start(out=sinb[:, csl], in_=sind[:, csl])
    for i in range(4):
        nc.gpsimd.dma_start(out=wo_sb[i], in_=woT[i * 128:(i + 1) * 128, :])
    nc.gpsimd.dma_start(out=tri, in_=trid)
    for st in range(16):
        v3 = vt[st].rearrange("p (h c) -> p h c", h=8)
        nc.gpsimd.memset(v3[:, :, 64:65], 1.0)

    # ---- pools --------------------------------------------------------------
    tmp_pool = ctx.enter_context(tc.tile_pool(name="tmp", bufs=2))
    pt_pool = ctx.enter_context(tc.tile_pool(name="pt", bufs=8))
    norm_pool = ctx.enter_context(tc.tile_pool(name="norm", bufs=2))
    ost_pool = ctx.enter_context(tc.tile_pool(name="ost", bufs=3))
    big_ps = ctx.enter_context(tc.tile_pool(name="big_ps", bufs=2, space="PSUM"))
    po_ps = ctx.enter_context(tc.tile_pool(name="po_ps", bufs=2, space="PSUM"))

    cos3 = cosb.rearrange("p (j c) -> p j c", j=2)
    sin3 = sinb.rearrange("p (j c) -> p j c", j=2)
    tri3 = tri.rearrange("p (j c) -> p j c", j=2)

    def r3(ap):
        return ap.rearrange("p (j c) -> p j c", j=2)

    def qk_proj_rope(ot, sc, xt):
        """Project q and k for head-pair ot of chunk sc, apply RoPE (bf16)."""
        ssl = slice(sc * 512, (sc + 1) * 512)
        ps = big_ps.tile([128, 1024], F32, tag="big", name=f"psqk{sc}_{ot}")
        osl = slice(ot * 128, (ot + 1) * 128)
        for ic in range(8):
            nc.tensor.matmul(ps[:, 0:512], wq_sb[ic][:, osl], xt[ic][:],
                             start=(ic == 0), stop=(ic == 7))
            nc.tensor.matmul(ps[:, 512:1024], wk_sb[ic][:, osl], xt[ic][:],
                             start=(ic == 0), stop=(ic == 7))
        xb = tmp_pool.tile([128, 1024], BF16, tag="xb", name="xb")
        nc.vector.tensor_copy(out=xb, in_=ps)
        xs = tmp_pool.tile([128, 1024], BF16, tag="xs", name="xs")
        nc.vector.stream_shuffle(xs[:], xb[:], SWAP_MASK)
        t1 = tmp_pool.tile([128, 1024], BF16, tag="t1", name="t1")
        nc.vector.tensor_tensor(r3(t1), r3(xb), cos3[:, :, ssl], MUL)
        t2 = tmp_pool.tile([128, 1024], BF16, tag="t2", name="t2")
        nc.vector.tensor_tensor(r3(t2), r3(xs), sin3[:, :, ssl], MUL)
        qk_dst = r3(qkT[ot])[:, :, ssl]
        nc.vector.tensor_tensor(qk_dst, r3(t1), r3(t2), ADD)

    def v_proj(xt, sc, on_scalar):
        """v-projection for chunk sc, ic-major so it streams as DMAs land."""
        for half in range(2):
            psv = big_ps.tile([128, 1024], F32, tag="big", name=f"psv{sc}_{half}")
            for ic in range(8):
                for sub in range(2):
                    stl = 2 * half + sub
                    nc.tensor.matmul(psv[:, sub * 512:(sub + 1) * 512],
                                     xt[ic][:, stl * 128:(stl + 1) * 128],
                                     wv_sb[ic][:],
                                     start=(ic == 0), stop=(ic == 7))
            for sub in range(2):
                st = 4 * sc + 2 * half + sub
                v3 = vt[st].rearrange("p (h c) -> p h c", h=8)
                p3 = psv[:, sub * 512:(sub + 1) * 512].rearrange(
                    "p (h c) -> p h c", h=8)
                if on_scalar:
                    nc.scalar.copy(out=v3[:, :, 0:64], in_=p3[:, :, :])
                else:
                    nc.vector.tensor_copy(out=v3[:, :, 0:64], in_=p3[:, :, :])

    def attn(hp, qc, po, kts, first_kt):
        """Score+softmax+PV for head-pair hp, query chunk qc, key tiles kts.

        Valid region is left-aligned: head A scores at cols [0:512-lo],
        head B at [512:1024-lo]; PV writes po cols [lo:512] / [512+lo:1024].
        """
        for kt in kts:
            d = kt - 4 * qc
            lo = 128 * d if d >= 1 else 0
            ksl = slice(S + kt * 128, S + (kt + 1) * 128)
            qsl = slice(qc * 512 + lo, (qc + 1) * 512)
            ps2 = big_ps.tile([128, 1024], F32, tag="big", name="ps2")
            with tc.high_priority(offset=500):
                nc.tensor.matmul(ps2[:, 0:512 - lo], qkT[hp][0:64, ksl],
                                 qkT[hp][0:64, qsl], start=True, stop=True)
                nc.tensor.matmul(ps2[:, 512:1024 - lo], qkT[hp][64:128, ksl],
                                 qkT[hp][64:128, qsl], start=True, stop=True)
                pt = pt_pool.tile([128, 1024], BF16, tag="pt", name="pt")
                nc.scalar.activation(pt[:, 0:1024 - lo], ps2[:, 0:1024 - lo],
                                     mybir.ActivationFunctionType.Exp, scale=0.125)
                if d >= 0:
                    ptd = r3(pt)[:, :, 0:128]
                    nc.vector.tensor_tensor(ptd, ptd, tri3[:, :, :], MUL)
            c0 = (2 * hp) * 65
            c1 = (2 * hp + 1) * 65
            st = (kt == first_kt)
            sp = (kt == kts[-1] and kt == 4 * qc + 3)
            nc.tensor.matmul(po[0:65, lo:512], vt[kt][:, c0:c0 + 65],
                             pt[:, 0:512 - lo], start=st, stop=sp)
            nc.tensor.matmul(po[0:65, 512 + lo:1024], vt[kt][:, c1:c1 + 65],
                             pt[:, 512:1024 - lo], start=st, stop=sp)

    def finish(hp, qc, po):
        """Evict both heads' PV accumulators, normalize by the ones-row.

        Head B (destined for oT partitions 64:128 via a shift DMA) is
        normalized first so its longer chain starts as early as possible.
        """
        qsl = slice(qc * 512, (qc + 1) * 512)
        otAB = norm_pool.tile([128, 1024], BF16, tag="otAB", name="otAB")
        nc.vector.tensor_copy(out=otAB[0:65, :], in_=po[0:65, :])
        # spread the 1024 l values over 128 partitions for the reciprocal
        lsp = norm_pool.tile([128, 8], BF16, tag="lsp", name="lsp")
        nc.sync.dma_start(out=lsp[:, :], in_=otAB[64:65, :])
        lspr = norm_pool.tile([128, 8], F32, tag="lspr", name="lspr")
        nc.vector.reciprocal(lspr[:, :], lsp[:, :])
        lb = norm_pool.tile([128, 1024], F32, tag="lb", name="lb")
        nc.sync.dma_start(out=lb[0:1, :], in_=lspr[:, :])
        nc.gpsimd.partition_broadcast(lb[0:64, :], lb[0:1, :], 64)
        o1 = norm_pool.tile([128, 512], BF16, tag="o1", name="o1")
        nc.vector.tensor_tensor(o1[0:64, :], otAB[0:64, 512:1024],
                                lb[0:64, 512:1024], MUL)
        nc.sync.dma_start(out=oT[hp][64:128, qsl], in_=o1[0:64, :])
        nc.vector.tensor_tensor(oT[hp][0:64, qsl], otAB[0:64, 0:512],
                                lb[0:64, 0:512], MUL)

    def store_ost(st, ost, split):
        stsl = slice(st * 128, (st + 1) * 128)
        if not split:
            nc.sync.dma_start(out=outp[stsl, :], in_=ost[:])
        else:
            # split across two queues to shorten the final drain
            nc.sync.dma_start(out=outp[st * 128:st * 128 + 64, :],
                              in_=ost[0:64, :])
            nc.scalar.dma_start(out=outp[st * 128 + 64:(st + 1) * 128, :],
                                in_=ost[64:128, :])

    def outproj_tile(qc, stl):
        """Output projection for one 128-row tile of chunk qc (all 4 hps)."""
        st = 4 * qc + stl
        stsl = slice(st * 128, (st + 1) * 128)
        pso = po_ps.tile([128, 1024], F32, tag="po", name="pso")
        for oc in range(2):
            osl = slice(oc * 512, (oc + 1) * 512)
            for hp in range(4):
                nc.tensor.matmul(pso[:, osl], oT[hp][:, stsl],
                                 wo_sb[hp][:, osl],
                                 start=(hp == 0), stop=(hp == 3))
        ost = ost_pool.tile([128, 1024], BF16, tag="ost", name="ost")
        nc.vector.tensor_copy(out=ost, in_=pso)
        store_ost(st, ost, split=False)

    def outproj_tail():
        """Chunk-3 output projection: hp0..2 partial sums land in the now-free
        score/po PSUM slots while finish(3) drains; hp3 then extends each
        accumulation group, one eviction + two split stores per tile."""
        qc = 3
        psos = []
        for stl in range(4):
            st = 4 * qc + stl
            stsl = slice(st * 128, (st + 1) * 128)
            pool = big_ps if stl < 2 else po_ps
            tag = "big" if stl < 2 else "po"
            pso = pool.tile([128, 1024], F32, tag=tag, name=f"psot{stl}")
            for oc in range(2):
                osl = slice(oc * 512, (oc + 1) * 512)
                for hp in range(3):
                    nc.tensor.matmul(pso[:, osl], oT[hp][:, stsl],
                                     wo_sb[hp][:, osl],
                                     start=(hp == 0), stop=False)
            psos.append(pso)
        for stl in range(4):
            st = 4 * qc + stl
            stsl = slice(st * 128, (st + 1) * 128)
            pso = psos[stl]
            for oc in range(2):
                osl = slice(oc * 512, (oc + 1) * 512)
                nc.tensor.matmul(pso[:, osl], oT[3][:, stsl],
                                 wo_sb[3][:, osl], start=False, stop=True)
            ost = ost_pool.tile([128, 1024], BF16, tag="ost3",
                                name=f"ost3_{stl}", bufs=4)
            nc.vector.tensor_copy(out=ost, in_=pso)
            store_ost(st, ost, split=True)

    # ---- chunk loop ---------------------------------------------------------
    # outproj(qc) is deferred into chunk qc+1, one st-tile after each hp's
    # finish, so the in-order PE stream never blocks on the normalization
    # chains and the exp-paced attention stretches get PE filler work.
    prefetched = [None]
    for sc in range(4):
        qc = sc
        nkt = 4 * qc + 4
        if sc == 0:
            xt = xt0
        elif sc == 1:
            xt = xt1
        else:
            xt = prefetched[0]

        if sc == 0:
            v_proj(xt0, 0, on_scalar=True)
            qk_proj_rope(0, 0, xt0)
            v_proj(xt1, 1, on_scalar=True)
            for ot in range(1, 4):
                qk_proj_rope(ot, 0, xt0)
            for hp in range(4):
                po = po_ps.tile([128, 1024], F32, tag="po", name="po")
                attn(hp, 0, po, list(range(4)), 0)
                finish(hp, 0, po)
        else:
            qk_proj_rope(0, sc, xt)
            qk_proj_rope(1, sc, xt)
            if sc >= 2:
                v_proj(xt, sc, on_scalar=False)
            po = po_ps.tile([128, 1024], F32, tag="po", name="po")
            attn(0, qc, po, list(range(4 * qc)), 0)
            qk_proj_rope(2, sc, xt)
            if sc < 3:
                nxt = []
                for ic in range(8):
                    t = xt_pool.tile([128, 512], BF16, tag=f"xt{ic}",
                                     name=f"xtp{sc + 1}_{ic}")
                    nc.sync.dma_start(
                        out=t, in_=xT[ic * 128:(ic + 1) * 128,
                                      (sc + 1) * 512:(sc + 2) * 512])
                    nxt.append(t)
                prefetched[0] = nxt
            attn(0, qc, po, list(range(4 * qc, nkt)), 0)
            finish(0, qc, po)
            outproj_tile(qc - 1, 0)
            qk_proj_rope(3, sc, xt)
            po = po_ps.tile([128, 1024], F32, tag="po", name="po")
            attn(1, qc, po, list(range(nkt)), 0)
            finish(1, qc, po)
            outproj_tile(qc - 1, 1)
            for hp in range(2, 4):
                po = po_ps.tile([128, 1024], F32, tag="po", name="po")
                attn(hp, qc, po, list(range(nkt)), 0)
                finish(hp, qc, po)
                outproj_tile(qc - 1, hp)
    outproj_tail()

    ctx.close()


_NC_CACHE = []
LAST_RESULT = None


def _get_program():
    if not _NC_CACHE:
        _NC_CACHE.append(_build_program())
    return _NC_CACHE[0]


def _host_tables(pos):
    p = np.arange(128)
    inv = (THETA ** (-2.0 * ((p % 64) // 2) / DH)).astype(np.float64)
    ang = pos.astype(np.float64)[None, :] * inv[:, None]          # [128, S]
    altsign = np.where(p % 2 == 0, -1.0, 1.0)[:, None]
    cosT = np.cos(ang)
    sinT = np.sin(ang) * altsign
    cos2 = np.concatenate([cosT, cosT], axis=1)                   # [128, 2S]
    sin2 = np.concatenate([sinT, sinT], axis=1)
    tri = np.zeros((128, 256), np.float32)
    c = np.arange(128)
    tri[:, 0:128] = (p[:, None] <= c[None, :])
    tri[:, 128:256] = tri[:, 0:128]
    return _bf16(cos2), _bf16(sin2), _bf16(tri)


def _bf16(a):
    import ml_dtypes
    return np.ascontiguousarray(np.asarray(a, dtype=np.float32)).astype(ml_dtypes.bfloat16)


def kernel(x, token_positions, wq, wk, wv, wo):
    x = np.asarray(x, dtype=np.float32)
    pos = np.asarray(token_positions, dtype=np.int32)
    wq = np.asarray(wq, dtype=np.float32)
    wk = np.asarray(wk, dtype=np.float32)
    wv = np.asarray(wv, dtype=np.float32)
    wo = np.asarray(wo, dtype=np.float32)

    nc = _get_program()
    cos2, sin2, tri = _host_tables(pos)

    in_maps = []
    for c in range(8):
        b, g = c // 2, c % 2
        gsl = slice(g * GD, (g + 1) * GD)
        in_maps.append({
            "xT": _bf16(x[b].T),
            "wqT": _bf16(wq.T[:, gsl]),
            "wkT": _bf16(wk.T[:, gsl]),
            "wvT": _bf16(wv.T[:, gsl]),
            "woT": _bf16(wo.T[gsl, :]),
            "cosd": cos2,
            "sind": sin2,
            "trid": tri,
        })

    old_m = nc.m
    nc.m = get_hw_module(nc.m)
    try:
        res = run_bass_kernel_spmd(nc, in_maps, core_ids=list(range(8)))
    finally:
        nc.m = old_m
    global LAST_RESULT
    LAST_RESULT = res

    out = np.empty((B, S, D), dtype=np.float32)
    for b in range(B):
        # tensor-parallel gather: sum the two head-group partials per batch
        out[b] = (res.results[2 * b]["outp"].astype(np.float32)
                  + res.results[2 * b + 1]["outp"].astype(np.float32))
    return out
